# revision 18
# baseline (speedup 1.0000x reference)
"""Trainium2 Bass kernel for BaseAttnPredictNet (pre-LN multi-head attention
with zero-attn slot, gated output combination, residual).

Sharding: data-parallel over (batch, query-rows). 8 cores, each takes 512
query rows (cores 0-3 -> batch 0, cores 4-7 -> batch 1). Each core
redundantly computes the K/V projections for its batch; no collectives.

On-device layout is "transposed world": activations as [feature, row]
(feature on partitions) so every matmul is out = W.T @ xT; the only
activation transposes are the LN outputs (PE transposes, bf16).

Softmax: no max-subtraction (scores are ~N(0,1), bounded); key mask folded
into the exp as a per-partition bias (0 or -1e30); denominator via an
appended ones-column in the PV matmul; LN gamma folded into the weights,
LN beta (if nonzero) via projected bias terms.
"""

import numpy as np

import concourse.bass as bass
import concourse.bacc as bacc
import concourse.mybir as mybir
import concourse.tile as tile
from concourse.bass_utils import run_bass_kernel_spmd
from concourse.masks import make_identity

# problem shapes (hardcoded per contract)
B, Q, KLEN, D = 2, 2048, 2048, 512
H, DH = 8, 64
P = 128
KP = 2176  # padded key length: 2048 real + 1 zero-attn slot + 127 masked pad
NJ = KP // P  # 17 key blocks
QS = 512  # query rows per core
NI = QS // P  # 4 row blocks
ND = D // P  # 4 feature blocks
NG = 2 * D // P  # 8 gate-contraction blocks
NCORES = 8
SCALE = 0.125
LN_EPS = 1e-5
NEGBIG = -1e30

F32 = mybir.dt.float32
BF16 = mybir.dt.bfloat16
AF = mybir.ActivationFunctionType
OP = mybir.AluOpType


def _build(use_gamma: bool, use_beta: bool) -> bass.Bass:
    nc = bacc.Bacc("TRN2", target_bir_lowering=False, debug=False)

    din = {}
    for name, shape in (
        ("q", [QS, D]),
        ("k", [KP, D]),
        ("v", [KP, D]),
        ("wq", [D, D]),
        ("wk", [D, D]),
        ("wv", [D, D]),
        ("wo", [D, D]),
        ("gw", [2 * D, D]),
        ("gb", [P, ND]),
        ("kmb", [P, NJ]),
        ("qm", [1, QS]),
    ):
        din[name] = nc.dram_tensor(name, shape, F32, kind="ExternalInput")
    if use_gamma:
        for name in ("qg", "kg", "vg"):
            din[name] = nc.dram_tensor(name, [P, ND], F32, kind="ExternalInput")
    if use_beta:
        for name in ("qb", "kb", "vb"):
            din[name] = nc.dram_tensor(name, [P, ND], F32, kind="ExternalInput")
    out_d = nc.dram_tensor("out", [QS, D], F32, kind="ExternalOutput")

    with tile.TileContext(nc) as tc:
        _body(nc, tc, din, out_d, use_gamma, use_beta)
    nc.compile()
    return nc


def _body(nc, tc, din, out_d, use_gamma, use_beta):
    from contextlib import ExitStack

    ctx = ExitStack()
    with ctx:
        persist = ctx.enter_context(tc.tile_pool(name="persist", bufs=1))
        stats = ctx.enter_context(tc.tile_pool(name="stats", bufs=6))
        # PSUM pools: exactly 8 banks total
        ptrans = ctx.enter_context(tc.tile_pool(name="ptrans", bufs=2, space="PSUM"))
        pproj = ctx.enter_context(tc.tile_pool(name="pproj", bufs=2, space="PSUM"))
        pS = ctx.enter_context(tc.tile_pool(name="pS", bufs=2, space="PSUM"))
        pav = ctx.enter_context(tc.tile_pool(name="pav", bufs=2, space="PSUM"))

        ident_bf = persist.tile([P, P], BF16)
        make_identity(nc, ident_bf)
        ident_f32 = persist.tile([P, P], F32)
        make_identity(nc, ident_f32)
        eps_t = persist.tile([P, 1], F32)
        nc.vector.memset(eps_t, LN_EPS)
        kmb = persist.tile([P, NJ], F32)
        nc.gpsimd.dma_start(out=kmb, in_=din["kmb"][:, :])
        qm_bc = persist.tile([P, QS], F32)
        _qm_ap = din["qm"][:, :]
        nc.gpsimd.dma_start(
            out=qm_bc,
            in_=bass.AP(tensor=_qm_ap.tensor, offset=_qm_ap.offset, ap=[[0, P], [1, QS]]),
        )
        gb = persist.tile([P, ND], F32)
        nc.gpsimd.dma_start(out=gb, in_=din["gb"][:, :])

        gam = {}
        if use_gamma:
            for nm in ("qg", "kg", "vg"):
                g = persist.tile([P, ND], F32, name=nm)
                nc.gpsimd.dma_start(out=g, in_=din[nm][:, :])
                gam[nm] = g
        bet = {}
        if use_beta:
            for nm in ("qb", "kb", "vb"):
                bt = persist.tile([P, ND], F32, name=nm)
                nc.gpsimd.dma_start(out=bt, in_=din[nm][:, :])
                bet[nm] = bt

        # xstage first: DMA-destination memory must never sit on recycled
        # pool space (walrus caps DMA waits at 2; recycled regions accumulate
        # cross-lane deps that Tile will not prune transitively)
        # bufs matched to the 8 SWDGE sem lanes: same-slot DMA predecessors
        # then share one lane, keeping every DMA at <=2 encoded waits.
        xstage = tc.alloc_tile_pool(name="xstage", bufs=8)
        # ---- weights: DMA per matrix-row-block into staging, cast to bf16 ----
        wstage = tc.alloc_tile_pool(name="wstage", bufs=8)
        w_bf = {}
        bxT = {}  # per-matrix (beta @ W)^T as [128, ND] (partition-major over n)
        for wname, gname, bname in (
            ("wq", "qg", "qb"),
            ("wk", "kg", "kb"),
            ("wv", "vg", "vb"),
            ("wo", None, None),
            ("gw", None, None),
        ):
            nblk = NG if wname == "gw" else ND
            wb = persist.tile([P, nblk, D], BF16, name=f"{wname}_bf")
            for b in range(nblk):
                wf = wstage.tile([P, D], F32, name="wstage_t")
                nc.gpsimd.dma_start(out=wf, in_=din[wname][b * P : (b + 1) * P, :])
                if use_gamma and gname is not None:
                    nc.vector.tensor_scalar(
                        out=wb[:, b, :],
                        in0=wf,
                        scalar1=gam[gname][:, b : b + 1],
                        scalar2=None,
                        op0=OP.mult,
                    )
                else:
                    nc.vector.tensor_copy(wb[:, b, :], wf)
            w_bf[wname] = wb
            if use_beta and bname is not None:
                # bxT [n,1] per n-blk: lhsT=W [d, n-blk], rhs=beta [d,1]
                bx = persist.tile([P, ND], F32, name=f"bx_{wname}")
                betb = persist.tile([P, ND], BF16, name=f"betb_{wname}")
                nc.vector.tensor_copy(betb, bet[bname])
                for a in range(ND):
                    pb = pproj.tile([P, 1], F32, name="pproj_t")
                    for b in range(nblk):
                        nc.tensor.matmul(
                            pb,
                            wb[:, b, a * P : (a + 1) * P],
                            betb[:, b : b + 1],
                            start=(b == 0),
                            stop=(b == nblk - 1),
                        )
                    nc.vector.tensor_copy(bx[:, a : a + 1], pb)
                bxT[wname] = bx
        wstage.release()

        # ---- persistent activation tensors ----
        qT_f = persist.tile([P, ND, QS], F32)
        qhT = persist.tile([P, ND, QS], BF16)
        khT = persist.tile([P, ND, KP], BF16)
        vh_aug = persist.tile([P, NJ, H, DH + 1], BF16)
        av_nat = persist.tile([P, NI, D], BF16)
        avT = persist.tile([P, ND, QS], BF16)
        poT_f = persist.tile([P, ND, QS], F32)
        gT = persist.tile([P, ND, QS], F32)
        outT = persist.tile([P, ND, QS], F32)

        def ln_scales(xf):
            """negmean and rstd for LN of a [128, D] f32 tile slice."""
            st = stats.tile([P, 6], F32, name="bnst")
            nc.vector.bn_stats(out=st, in_=xf)
            mv = stats.tile([P, 2], F32, name="bnagg")
            nc.vector.bn_aggr(out=mv, in_=st)
            std = stats.tile([P, 1], F32, name="std")
            nc.scalar.activation(out=std, in_=mv[:, 1:2], func=AF.Sqrt, bias=eps_t)
            rstd = stats.tile([P, 1], F32, name="rstd")
            nc.vector.reciprocal(rstd, std)
            negm = stats.tile([P, 1], F32, name="negm")
            nc.vector.tensor_scalar_mul(negm, mv[:, 0:1], -1.0)
            return negm, rstd

        def load_chunk(src_dram, r0, cw):
            """DMA rows [r0*P, (r0+cw)*P) as one transfer -> [P, cw, D]."""
            xf = xstage.tile([P, 2, D], F32, name="xstage_t")
            src = din[src_dram][r0 * P : (r0 + cw) * P, :].rearrange(
                "(c p) d -> p c d", p=P
            )
            nc.gpsimd.dma_start(out=xf[:, :cw, :], in_=src)
            return xf

        def ln_transpose(src_dram, nrows, xnT_dest):
            """Stream rows: LN (DVE) -> bf16 -> PE-transpose into
            xnT_dest [P, ND, nrows]."""
            nblk = nrows // P
            for c0 in range(0, nblk, 2):
                cw = min(2, nblk - c0)
                xf = load_chunk(src_dram, c0, cw)
                xn_chunk = []
                for cc in range(cw):
                    negm, rstd = ln_scales(xf[:, cc, :])
                    xn = stats.tile([P, D], BF16, name="xnorm")
                    nc.vector.tensor_scalar(
                        out=xn,
                        in0=xf[:, cc, :],
                        scalar1=negm,
                        scalar2=rstd,
                        op0=OP.add,
                        op1=OP.mult,
                    )
                    xn_chunk.append(xn)
                for b in range(ND):
                    pt = ptrans.tile([P, 2 * P], BF16, name="ptrans_t")
                    for cc in range(cw):
                        nc.tensor.transpose(
                            pt[:, cc * P : (cc + 1) * P],
                            xn_chunk[cc][:, b * P : (b + 1) * P],
                            ident_bf,
                        )
                    nc.vector.tensor_copy(
                        xnT_dest[:, b, c0 * P : (c0 + cw) * P], pt[:, : cw * P]
                    )

        pa_qk = tc.alloc_tile_pool(name="pa_qk", bufs=1)
        qnT = pa_qk.tile([P, ND, QS], BF16)
        knT = pa_qk.tile([P, ND, KP], BF16)

        # ---- q: raw transpose (f32 + bf16 casts) and LN transpose ----
        qf0 = load_chunk("q", 0, 2)
        qf1 = load_chunk("q", 2, 2)
        qparts = [qf0[:, 0, :], qf0[:, 1, :], qf1[:, 0, :], qf1[:, 1, :]]
        qcopy = xstage.tile([P, ND, D], F32, name="qcopy", bufs=1)
        for cc in range(NI):
            nc.vector.tensor_copy(qcopy[:, cc, :], qparts[cc])
        for b in range(ND):
            pt = ptrans.tile([P, 4 * P], F32, name="ptrans_t")
            for cc in range(NI):
                nc.tensor.transpose(
                    pt[:, cc * P : (cc + 1) * P],
                    qcopy[:, cc, b * P : (b + 1) * P],
                    ident_f32,
                )
            nc.vector.tensor_copy(qT_f[:, b, :], pt)
        qn_chunk = []
        for cc in range(NI):
            negm, rstd = ln_scales(qparts[cc])
            xn = stats.tile([P, D], BF16, name="xnorm")
            nc.vector.tensor_scalar(
                out=xn,
                in0=qparts[cc],
                scalar1=negm,
                scalar2=rstd,
                op0=OP.add,
                op1=OP.mult,
            )
            qn_chunk.append(xn)
        for b in range(ND):
            pt = ptrans.tile([P, 4 * P], BF16, name="ptrans_t")
            for cc in range(NI):
                nc.tensor.transpose(
                    pt[:, cc * P : (cc + 1) * P],
                    qn_chunk[cc][:, b * P : (b + 1) * P],
                    ident_bf,
                )
            nc.vector.tensor_copy(qnT[:, b, :], pt)

        # ---- k: LN + transpose ----
        ln_transpose("k", KP, knT)

        # ---- q/k projections ----
        # qhT [n, i] = Wq'.T @ qnT
        for a in range(ND):
            pp = pproj.tile([P, QS], F32, name="pproj_t")
            for b in range(ND):
                nc.tensor.matmul(
                    pp,
                    w_bf["wq"][:, b, a * P : (a + 1) * P],
                    qnT[:, b, :],
                    start=(b == 0),
                    stop=(b == ND - 1),
                )
            if use_beta:
                nc.vector.tensor_scalar(
                    out=qhT[:, a, :],
                    in0=pp,
                    scalar1=bxT["wq"][:, a : a + 1],
                    scalar2=None,
                    op0=OP.add,
                )
            else:
                nc.vector.tensor_copy(qhT[:, a, :], pp)
        # khT [n, j] = Wk'.T @ knT   (j in chunks of 512)
        for a in range(ND):
            for j0 in range(0, KP, 512):
                jw = min(512, KP - j0)
                pp = pproj.tile([P, QS], F32, name="pproj_t")
                for b in range(ND):
                    nc.tensor.matmul(
                        pp[:, :jw],
                        w_bf["wk"][:, b, a * P : (a + 1) * P],
                        knT[:, b, j0 : j0 + jw],
                        start=(b == 0),
                        stop=(b == ND - 1),
                    )
                if use_beta:
                    nc.vector.tensor_scalar(
                        out=khT[:, a, j0 : j0 + jw],
                        in0=pp[:, :jw],
                        scalar1=bxT["wk"][:, a : a + 1],
                        scalar2=None,
                        op0=OP.add,
                    )
                else:
                    nc.vector.tensor_copy(khT[:, a, j0 : j0 + jw], pp[:, :jw])
        pa_qk.release()

        # ---- v: LN + transpose, then vh ----
        pa_v = tc.alloc_tile_pool(name="pa_v", bufs=1)
        vnT = pa_v.tile([P, ND, KP], BF16)
        ln_transpose("v", KP, vnT)
        # vh natural [j, n] = vnT.T @ Wv', into vh_aug (65-strided heads)
        for c in range(NJ):
            pp = pproj.tile([P, QS], F32, name="pproj_t")
            for b in range(ND):
                nc.tensor.matmul(
                    pp,
                    vnT[:, b, c * P : (c + 1) * P],
                    w_bf["wv"][:, b, :],
                    start=(b == 0),
                    stop=(b == ND - 1),
                )
            pp3 = pp.rearrange("p (h e) -> p h e", h=H)
            nc.vector.tensor_copy(vh_aug[:, c, :, 0:DH], pp3)
            nc.vector.memset(vh_aug[:, c, :, DH : DH + 1], 1.0)
        pa_v.release()
        xstage.release()

        # ---- attention, head by head ----
        pb_attn = ctx.enter_context(tc.tile_pool(name="pb_attn", bufs=2))
        for h in range(H):
            nb = h // 2
            r0 = (h % 2) * DH
            expS = pb_attn.tile([P, NJ, QS], BF16, name="expS")
            for c in range(NJ):
                ps = pS.tile([P, QS], F32, name="pS_t")
                nc.tensor.matmul(
                    ps,
                    khT[r0 : r0 + DH, nb, c * P : (c + 1) * P],
                    qhT[r0 : r0 + DH, nb, :],
                    start=True,
                    stop=True,
                )
                nc.scalar.activation(
                    out=expS[:, c, :],
                    in_=ps,
                    func=AF.Exp,
                    bias=kmb[:, c : c + 1],
                    scale=SCALE,
                )
            for a in range(NI):
                pv = pav.tile([P, DH + 1], F32, name="pav_t")
                for c in range(NJ):
                    nc.tensor.matmul(
                        pv,
                        expS[:, c, a * P : (a + 1) * P],
                        vh_aug[:, c, h, :],
                        start=(c == 0),
                        stop=(c == NJ - 1),
                    )
                rden = stats.tile([P, 1], F32, name="rden")
                nc.vector.reciprocal(rden, pv[:, DH : DH + 1])
                nc.vector.tensor_scalar(
                    out=av_nat[:, a, h * DH : (h + 1) * DH],
                    in0=pv[:, 0:DH],
                    scalar1=rden,
                    scalar2=None,
                    op0=OP.mult,
                )

        # ---- avT (with query-mask fold; beta_v enters here since
        # sum(attn)=1 makes +bv commute with the softmax average) ----
        for b in range(ND):
            pt = ptrans.tile([P, 4 * P], BF16, name="ptrans_t")
            for a in range(NI):
                nc.tensor.transpose(
                    pt[:, a * P : (a + 1) * P],
                    av_nat[:, a, b * P : (b + 1) * P],
                    ident_bf,
                )
            if use_beta:
                tbv = pb_attn.tile([P, QS], BF16, name="tbv")
                nc.vector.tensor_scalar(
                    out=tbv, in0=pt, scalar1=bxT["wv"][:, b : b + 1],
                    scalar2=None, op0=OP.add,
                )
                nc.vector.tensor_tensor(out=avT[:, b, :], in0=tbv, in1=qm_bc, op=OP.mult)
            else:
                nc.vector.tensor_tensor(out=avT[:, b, :], in0=pt, in1=qm_bc, op=OP.mult)

        # ---- output projection poT = Wo.T @ avT ----
        for a in range(ND):
            pp = pproj.tile([P, QS], F32, name="pproj_t")
            for b in range(ND):
                nc.tensor.matmul(
                    pp,
                    w_bf["wo"][:, b, a * P : (a + 1) * P],
                    avT[:, b, :],
                    start=(b == 0),
                    stop=(b == ND - 1),
                )
            nc.vector.tensor_copy(poT_f[:, a, :], pp)

        # ---- gate gT = sigmoid(gw.T @ [qT; poT] + gb) ----
        gate_rhs = []
        for b in range(NG):
            src = qT_f[:, b, :] if b < ND else poT_f[:, b - ND, :]
            cb = pb_attn.tile([P, QS], BF16, name="gatecast", bufs=8)
            nc.vector.tensor_copy(cb, src)
            gate_rhs.append(cb)
        for a in range(ND):
            pp = pproj.tile([P, QS], F32, name="pproj_t")
            for b in range(NG):
                rhs = gate_rhs[b]
                nc.tensor.matmul(
                    pp,
                    w_bf["gw"][:, b, a * P : (a + 1) * P],
                    rhs,
                    start=(b == 0),
                    stop=(b == NG - 1),
                )
            nc.scalar.activation(
                out=gT[:, a, :], in_=pp, func=AF.Sigmoid, bias=gb[:, a : a + 1]
            )

        # ---- final combine: out = q + po + g*(q - po) ----
        for a in range(ND):
            s = pb_attn.tile([P, QS], F32, name="fin_t", bufs=6)
            nc.vector.tensor_tensor(
                out=s, in0=qT_f[:, a, :], in1=poT_f[:, a, :], op=OP.subtract
            )
            m = pb_attn.tile([P, QS], F32, name="fin_t", bufs=6)
            nc.vector.tensor_tensor(out=m, in0=gT[:, a, :], in1=s, op=OP.mult)
            r = pb_attn.tile([P, QS], F32, name="fin_t", bufs=6)
            nc.vector.tensor_tensor(
                out=r, in0=qT_f[:, a, :], in1=poT_f[:, a, :], op=OP.add
            )
            nc.vector.tensor_tensor(out=outT[:, a, :], in0=m, in1=r, op=OP.add)

        # ---- transpose back + one combined store ----
        out_nat = persist.tile([P, NI, D], F32, name="outn")
        for a in range(NI):
            pt = ptrans.tile([P, 4 * P], F32, name="ptrans_t")
            for b in range(ND):
                nc.tensor.transpose(
                    pt[:, b * P : (b + 1) * P],
                    outT[:, b, a * P : (a + 1) * P],
                    ident_f32,
                )
            nc.vector.tensor_copy(out_nat[:, a, :], pt)
        dst = out_d[:, :].rearrange("(a p) d -> p a d", p=P)
        nc.gpsimd.dma_start(out=dst, in_=out_nat)


_CACHE: dict = {}


def make_in_maps(inputs):
    """Shard full inputs into per-core input maps; returns (in_maps, flags)."""
    q = np.asarray(inputs["query"], np.float32)
    k = np.asarray(inputs["key"], np.float32)
    v = np.asarray(inputs["value"], np.float32)
    wq = np.asarray(inputs["weight_q"], np.float32)
    wk = np.asarray(inputs["weight_k"], np.float32)
    wv = np.asarray(inputs["weight_v"], np.float32)
    wo = np.asarray(inputs["weight_o"], np.float32)
    gw = np.asarray(inputs["g_w"], np.float32)
    gb = np.asarray(inputs["g_b"], np.float32)
    qmask = np.asarray(inputs["query_mask"])
    kmask = np.asarray(inputs["key_mask"])
    gams = [
        np.asarray(inputs[n], np.float32) for n in ("q_gamma", "k_gamma", "v_gamma")
    ]
    bets = [np.asarray(inputs[n], np.float32) for n in ("q_beta", "k_beta", "v_beta")]

    use_gamma = any(not np.allclose(g, 1.0) for g in gams)
    use_beta = any(np.any(bt != 0.0) for bt in bets)

    def colmajor(vec):  # [D] -> [128, ND] partition-major
        return np.ascontiguousarray(vec.reshape(-1, P).T)

    # padded K/V + per-key exp bias (0 = attend, -1e30 = masked)
    kpad = np.zeros((B, KP, D), np.float32)
    vpad = np.zeros((B, KP, D), np.float32)
    kpad[:, :KLEN] = k
    vpad[:, :KLEN] = v
    kmb = np.full((B, KP), NEGBIG, np.float32)
    kmb[:, :KLEN] = np.where(kmask == 0, NEGBIG, 0.0)
    kmb[:, KLEN] = 0.0  # zero-attn slot always attendable

    per_batch = NCORES // B
    in_maps = []
    for c in range(NCORES):
        b, r = c // per_batch, c % per_batch
        m = {
            "q": np.ascontiguousarray(q[b, r * QS : (r + 1) * QS]),
            "k": kpad[b],
            "v": vpad[b],
            "wq": wq,
            "wk": wk,
            "wv": wv,
            "wo": wo,
            "gw": gw,
            "gb": colmajor(gb),
            "kmb": np.ascontiguousarray(kmb[b].reshape(NJ, P).T),
            "qm": qmask[b, r * QS : (r + 1) * QS].astype(np.float32)[None, :],
        }
        if use_gamma:
            m["qg"], m["kg"], m["vg"] = (colmajor(g) for g in gams)
        if use_beta:
            m["qb"], m["kb"], m["vb"] = (colmajor(bt) for bt in bets)
        in_maps.append(m)
    return in_maps, (use_gamma, use_beta)


def kernel(_return_res=False, _run_kwargs=None, **inputs):
    run_kwargs = _run_kwargs or {}
    in_maps, key = make_in_maps(inputs)
    if key not in _CACHE:
        _CACHE[key] = _build(*key)
    nc = _CACHE[key]
    res = run_bass_kernel_spmd(nc, in_maps, list(range(NCORES)), **run_kwargs)
    out = np.empty((B, Q, D), np.float32)
    per_batch = NCORES // B
    for c in range(NCORES):
        b, r = c // per_batch, c % per_batch
        out[b, r * QS : (r + 1) * QS] = res.results[c]["out"]
    if _return_res:
        return out, res
    return out


# revision 20
# speedup vs baseline: 45.5773x; 45.5773x over previous
"""Trainium2 Bass kernel for BaseAttnPredictNet (pre-LN multi-head attention
with zero-attn slot, gated output combination, residual).

Sharding: data-parallel over (batch, query-rows). 8 cores, each takes 512
query rows (cores 0-3 -> batch 0, cores 4-7 -> batch 1). Each core
redundantly computes the K/V projections for its batch; no collectives.

On-device layout is "transposed world": activations as [feature, row]
(feature on partitions) so every matmul is out = W.T @ xT; the only
activation transposes are the LN outputs (PE transposes, bf16).

Softmax: no max-subtraction (scores are ~N(0,1), bounded); key mask folded
into the exp as a per-partition bias (0 or -1e30); denominator via an
appended ones-column in the PV matmul; LN gamma folded into the weights,
LN beta (if nonzero) via projected bias terms.
"""

import numpy as np

import concourse.bass as bass
import concourse.bacc as bacc
import concourse.mybir as mybir
import concourse.tile as tile
from concourse.bass_utils import run_bass_kernel_spmd
from concourse.masks import make_identity

# problem shapes (hardcoded per contract)
B, Q, KLEN, D = 2, 2048, 2048, 512
H, DH = 8, 64
P = 128
KP = 2176  # padded key length: 2048 real + 1 zero-attn slot + 127 masked pad
NJ = KP // P  # 17 key blocks
QS = 512  # query rows per core
NI = QS // P  # 4 row blocks
ND = D // P  # 4 feature blocks
NG = 2 * D // P  # 8 gate-contraction blocks
NCORES = 8
SCALE = 0.125
LN_EPS = 1e-5
NEGBIG = -1e30

F32 = mybir.dt.float32
BF16 = mybir.dt.bfloat16
AF = mybir.ActivationFunctionType
OP = mybir.AluOpType


def _build(use_gamma: bool, use_beta: bool, reps: int = 1) -> bass.Bass:
    """reps>1 unrolls the whole body N times (same I/O) for delta-timing."""
    nc = bacc.Bacc("TRN2", target_bir_lowering=False, debug=False)

    din = {}
    for name, shape in (
        ("q", [QS, D]),
        ("k", [KP, D]),
        ("v", [KP, D]),
        ("wq", [D, D]),
        ("wk", [D, D]),
        ("wv", [D, D]),
        ("wo", [D, D]),
        ("gw", [2 * D, D]),
        ("gb", [P, ND]),
        ("kmb", [P, NJ]),
        ("qm", [1, QS]),
    ):
        din[name] = nc.dram_tensor(name, shape, F32, kind="ExternalInput")
    if use_gamma:
        for name in ("qg", "kg", "vg"):
            din[name] = nc.dram_tensor(name, [P, ND], F32, kind="ExternalInput")
    if use_beta:
        for name in ("qb", "kb", "vb"):
            din[name] = nc.dram_tensor(name, [P, ND], F32, kind="ExternalInput")
    out_d = nc.dram_tensor("out", [QS, D], F32, kind="ExternalOutput")

    with tile.TileContext(nc) as tc:
        for i in range(reps):
            # chain reps through the output tensor so DCE keeps every copy
            q_src = din["q"] if i == 0 else out_d
            _body(nc, tc, din, out_d, use_gamma, use_beta, q_src=q_src)
    nc.compile()
    return nc


def _body(nc, tc, din, out_d, use_gamma, use_beta, q_src=None):
    if q_src is None:
        q_src = din["q"]
    from contextlib import ExitStack

    ctx = ExitStack()
    with ctx:
        persist = ctx.enter_context(tc.tile_pool(name="persist", bufs=1))
        stats = ctx.enter_context(tc.tile_pool(name="stats", bufs=6))
        # PSUM pools: exactly 8 banks total
        ptrans = ctx.enter_context(tc.tile_pool(name="ptrans", bufs=2, space="PSUM"))
        pproj = ctx.enter_context(tc.tile_pool(name="pproj", bufs=2, space="PSUM"))
        pS = ctx.enter_context(tc.tile_pool(name="pS", bufs=2, space="PSUM"))
        pav = ctx.enter_context(tc.tile_pool(name="pav", bufs=2, space="PSUM"))

        ident_bf = persist.tile([P, P], BF16)
        make_identity(nc, ident_bf)
        ident_f32 = persist.tile([P, P], F32)
        make_identity(nc, ident_f32)
        eps_t = persist.tile([P, 1], F32)
        nc.vector.memset(eps_t, LN_EPS)
        kmb = persist.tile([P, NJ], F32)
        nc.gpsimd.dma_start(out=kmb, in_=din["kmb"][:, :])
        qm_bc = persist.tile([P, QS], F32)
        _qm_ap = din["qm"][:, :]
        nc.gpsimd.dma_start(
            out=qm_bc,
            in_=bass.AP(tensor=_qm_ap.tensor, offset=_qm_ap.offset, ap=[[0, P], [1, QS]]),
        )
        gb = persist.tile([P, ND], F32)
        nc.gpsimd.dma_start(out=gb, in_=din["gb"][:, :])

        gam = {}
        if use_gamma:
            for nm in ("qg", "kg", "vg"):
                g = persist.tile([P, ND], F32, name=nm)
                nc.gpsimd.dma_start(out=g, in_=din[nm][:, :])
                gam[nm] = g
        bet = {}
        if use_beta:
            for nm in ("qb", "kb", "vb"):
                bt = persist.tile([P, ND], F32, name=nm)
                nc.gpsimd.dma_start(out=bt, in_=din[nm][:, :])
                bet[nm] = bt

        # xstage first: DMA-destination memory must never sit on recycled
        # pool space (walrus caps DMA waits at 2; recycled regions accumulate
        # cross-lane deps that Tile will not prune transitively)
        # bufs matched to the 8 SWDGE sem lanes: same-slot DMA predecessors
        # then share one lane, keeping every DMA at <=2 encoded waits.
        xstage = tc.alloc_tile_pool(name="xstage", bufs=8)
        # ---- weights: DMA per matrix-row-block into staging, cast to bf16 ----
        wstage = tc.alloc_tile_pool(name="wstage", bufs=8)
        w_bf = {}
        bxT = {}  # per-matrix (beta @ W)^T as [128, ND] (partition-major over n)
        for wname, gname, bname in (
            ("wq", "qg", "qb"),
            ("wk", "kg", "kb"),
            ("wv", "vg", "vb"),
            ("wo", None, None),
            ("gw", None, None),
        ):
            nblk = NG if wname == "gw" else ND
            wb = persist.tile([P, nblk, D], BF16, name=f"{wname}_bf")
            for b in range(nblk):
                wf = wstage.tile([P, D], F32, name="wstage_t")
                nc.gpsimd.dma_start(out=wf, in_=din[wname][b * P : (b + 1) * P, :])
                if use_gamma and gname is not None:
                    nc.vector.tensor_scalar(
                        out=wb[:, b, :],
                        in0=wf,
                        scalar1=gam[gname][:, b : b + 1],
                        scalar2=None,
                        op0=OP.mult,
                    )
                else:
                    nc.vector.tensor_copy(wb[:, b, :], wf)
            w_bf[wname] = wb
            if use_beta and bname is not None:
                # bxT [n,1] per n-blk: lhsT=W [d, n-blk], rhs=beta [d,1]
                bx = persist.tile([P, ND], F32, name=f"bx_{wname}")
                betb = persist.tile([P, ND], BF16, name=f"betb_{wname}")
                nc.vector.tensor_copy(betb, bet[bname])
                for a in range(ND):
                    pb = pproj.tile([P, 1], F32, name="pproj_t")
                    for b in range(nblk):
                        nc.tensor.matmul(
                            pb,
                            wb[:, b, a * P : (a + 1) * P],
                            betb[:, b : b + 1],
                            start=(b == 0),
                            stop=(b == nblk - 1),
                        )
                    nc.vector.tensor_copy(bx[:, a : a + 1], pb)
                bxT[wname] = bx
        wstage.release()

        # ---- persistent activation tensors ----
        qT_f = persist.tile([P, ND, QS], F32)
        qhT = persist.tile([P, ND, QS], BF16)
        khT = persist.tile([P, ND, KP], BF16)
        vh_aug = persist.tile([P, NJ, H, DH + 1], BF16)
        av_nat = persist.tile([P, NI, D], BF16)
        avT = persist.tile([P, ND, QS], BF16)
        poT_f = persist.tile([P, ND, QS], F32)
        gT = persist.tile([P, ND, QS], F32)
        outT = persist.tile([P, ND, QS], F32)

        def ln_scales(xf):
            """negmean and rstd for LN of a [128, D] f32 tile slice."""
            st = stats.tile([P, 6], F32, name="bnst")
            nc.vector.bn_stats(out=st, in_=xf)
            mv = stats.tile([P, 2], F32, name="bnagg")
            nc.vector.bn_aggr(out=mv, in_=st)
            std = stats.tile([P, 1], F32, name="std")
            nc.scalar.activation(out=std, in_=mv[:, 1:2], func=AF.Sqrt, bias=eps_t)
            rstd = stats.tile([P, 1], F32, name="rstd")
            nc.vector.reciprocal(rstd, std)
            negm = stats.tile([P, 1], F32, name="negm")
            nc.vector.tensor_scalar_mul(negm, mv[:, 0:1], -1.0)
            return negm, rstd

        def load_chunk(src_dram, r0, cw):
            """DMA rows [r0*P, (r0+cw)*P) as one transfer -> [P, cw, D]."""
            xf = xstage.tile([P, 2, D], F32, name="xstage_t")
            base = q_src if src_dram == "q" else din[src_dram]
            src = base[r0 * P : (r0 + cw) * P, :].rearrange(
                "(c p) d -> p c d", p=P
            )
            nc.gpsimd.dma_start(out=xf[:, :cw, :], in_=src)
            return xf

        def ln_transpose(src_dram, nrows, xnT_dest):
            """Stream rows: LN (DVE) -> bf16 -> PE-transpose into
            xnT_dest [P, ND, nrows]."""
            nblk = nrows // P
            for c0 in range(0, nblk, 2):
                cw = min(2, nblk - c0)
                xf = load_chunk(src_dram, c0, cw)
                xn_chunk = []
                for cc in range(cw):
                    negm, rstd = ln_scales(xf[:, cc, :])
                    xn = stats.tile([P, D], BF16, name="xnorm")
                    nc.vector.tensor_scalar(
                        out=xn,
                        in0=xf[:, cc, :],
                        scalar1=negm,
                        scalar2=rstd,
                        op0=OP.add,
                        op1=OP.mult,
                    )
                    xn_chunk.append(xn)
                for b in range(ND):
                    pt = ptrans.tile([P, 2 * P], BF16, name="ptrans_t")
                    for cc in range(cw):
                        nc.tensor.transpose(
                            pt[:, cc * P : (cc + 1) * P],
                            xn_chunk[cc][:, b * P : (b + 1) * P],
                            ident_bf,
                        )
                    nc.vector.tensor_copy(
                        xnT_dest[:, b, c0 * P : (c0 + cw) * P], pt[:, : cw * P]
                    )

        pa_qk = tc.alloc_tile_pool(name="pa_qk", bufs=1)
        qnT = pa_qk.tile([P, ND, QS], BF16)
        knT = pa_qk.tile([P, ND, KP], BF16)

        # ---- q: raw transpose (f32 + bf16 casts) and LN transpose ----
        qf0 = load_chunk("q", 0, 2)
        qf1 = load_chunk("q", 2, 2)
        qparts = [qf0[:, 0, :], qf0[:, 1, :], qf1[:, 0, :], qf1[:, 1, :]]
        qcopy = xstage.tile([P, ND, D], F32, name="qcopy", bufs=1)
        for cc in range(NI):
            nc.vector.tensor_copy(qcopy[:, cc, :], qparts[cc])
        for b in range(ND):
            pt = ptrans.tile([P, 4 * P], F32, name="ptrans_t")
            for cc in range(NI):
                nc.tensor.transpose(
                    pt[:, cc * P : (cc + 1) * P],
                    qcopy[:, cc, b * P : (b + 1) * P],
                    ident_f32,
                )
            nc.vector.tensor_copy(qT_f[:, b, :], pt)
        qn_chunk = []
        for cc in range(NI):
            negm, rstd = ln_scales(qparts[cc])
            xn = stats.tile([P, D], BF16, name="xnorm")
            nc.vector.tensor_scalar(
                out=xn,
                in0=qparts[cc],
                scalar1=negm,
                scalar2=rstd,
                op0=OP.add,
                op1=OP.mult,
            )
            qn_chunk.append(xn)
        for b in range(ND):
            pt = ptrans.tile([P, 4 * P], BF16, name="ptrans_t")
            for cc in range(NI):
                nc.tensor.transpose(
                    pt[:, cc * P : (cc + 1) * P],
                    qn_chunk[cc][:, b * P : (b + 1) * P],
                    ident_bf,
                )
            nc.vector.tensor_copy(qnT[:, b, :], pt)

        # ---- k: LN + transpose ----
        ln_transpose("k", KP, knT)

        # ---- q/k projections ----
        # qhT [n, i] = Wq'.T @ qnT
        for a in range(ND):
            pp = pproj.tile([P, QS], F32, name="pproj_t")
            for b in range(ND):
                nc.tensor.matmul(
                    pp,
                    w_bf["wq"][:, b, a * P : (a + 1) * P],
                    qnT[:, b, :],
                    start=(b == 0),
                    stop=(b == ND - 1),
                )
            if use_beta:
                nc.vector.tensor_scalar(
                    out=qhT[:, a, :],
                    in0=pp,
                    scalar1=bxT["wq"][:, a : a + 1],
                    scalar2=None,
                    op0=OP.add,
                )
            else:
                nc.vector.tensor_copy(qhT[:, a, :], pp)
        # khT [n, j] = Wk'.T @ knT   (j in chunks of 512)
        for a in range(ND):
            for j0 in range(0, KP, 512):
                jw = min(512, KP - j0)
                pp = pproj.tile([P, QS], F32, name="pproj_t")
                for b in range(ND):
                    nc.tensor.matmul(
                        pp[:, :jw],
                        w_bf["wk"][:, b, a * P : (a + 1) * P],
                        knT[:, b, j0 : j0 + jw],
                        start=(b == 0),
                        stop=(b == ND - 1),
                    )
                if use_beta:
                    nc.vector.tensor_scalar(
                        out=khT[:, a, j0 : j0 + jw],
                        in0=pp[:, :jw],
                        scalar1=bxT["wk"][:, a : a + 1],
                        scalar2=None,
                        op0=OP.add,
                    )
                else:
                    nc.vector.tensor_copy(khT[:, a, j0 : j0 + jw], pp[:, :jw])
        pa_qk.release()

        # ---- v: LN + transpose, then vh ----
        pa_v = tc.alloc_tile_pool(name="pa_v", bufs=1)
        vnT = pa_v.tile([P, ND, KP], BF16)
        ln_transpose("v", KP, vnT)
        # vh natural [j, n] = vnT.T @ Wv', into vh_aug (65-strided heads)
        for c in range(NJ):
            pp = pproj.tile([P, QS], F32, name="pproj_t")
            for b in range(ND):
                nc.tensor.matmul(
                    pp,
                    vnT[:, b, c * P : (c + 1) * P],
                    w_bf["wv"][:, b, :],
                    start=(b == 0),
                    stop=(b == ND - 1),
                )
            pp3 = pp.rearrange("p (h e) -> p h e", h=H)
            nc.vector.tensor_copy(vh_aug[:, c, :, 0:DH], pp3)
            nc.vector.memset(vh_aug[:, c, :, DH : DH + 1], 1.0)
        pa_v.release()
        xstage.release()

        # ---- attention, head by head ----
        pb_attn = ctx.enter_context(tc.tile_pool(name="pb_attn", bufs=2))
        for h in range(H):
            nb = h // 2
            r0 = (h % 2) * DH
            expS = pb_attn.tile([P, NJ, QS], BF16, name="expS")
            for c in range(NJ):
                ps = pS.tile([P, QS], F32, name="pS_t")
                nc.tensor.matmul(
                    ps,
                    khT[r0 : r0 + DH, nb, c * P : (c + 1) * P],
                    qhT[r0 : r0 + DH, nb, :],
                    start=True,
                    stop=True,
                )
                nc.scalar.activation(
                    out=expS[:, c, :],
                    in_=ps,
                    func=AF.Exp,
                    bias=kmb[:, c : c + 1],
                    scale=SCALE,
                )
            for a in range(NI):
                pv = pav.tile([P, DH + 1], F32, name="pav_t")
                for c in range(NJ):
                    nc.tensor.matmul(
                        pv,
                        expS[:, c, a * P : (a + 1) * P],
                        vh_aug[:, c, h, :],
                        start=(c == 0),
                        stop=(c == NJ - 1),
                    )
                rden = stats.tile([P, 1], F32, name="rden")
                nc.vector.reciprocal(rden, pv[:, DH : DH + 1])
                nc.vector.tensor_scalar(
                    out=av_nat[:, a, h * DH : (h + 1) * DH],
                    in0=pv[:, 0:DH],
                    scalar1=rden,
                    scalar2=None,
                    op0=OP.mult,
                )

        # ---- avT (with query-mask fold; beta_v enters here since
        # sum(attn)=1 makes +bv commute with the softmax average) ----
        for b in range(ND):
            pt = ptrans.tile([P, 4 * P], BF16, name="ptrans_t")
            for a in range(NI):
                nc.tensor.transpose(
                    pt[:, a * P : (a + 1) * P],
                    av_nat[:, a, b * P : (b + 1) * P],
                    ident_bf,
                )
            if use_beta:
                tbv = pb_attn.tile([P, QS], BF16, name="tbv")
                nc.vector.tensor_scalar(
                    out=tbv, in0=pt, scalar1=bxT["wv"][:, b : b + 1],
                    scalar2=None, op0=OP.add,
                )
                nc.vector.tensor_tensor(out=avT[:, b, :], in0=tbv, in1=qm_bc, op=OP.mult)
            else:
                nc.vector.tensor_tensor(out=avT[:, b, :], in0=pt, in1=qm_bc, op=OP.mult)

        # ---- output projection poT = Wo.T @ avT ----
        for a in range(ND):
            pp = pproj.tile([P, QS], F32, name="pproj_t")
            for b in range(ND):
                nc.tensor.matmul(
                    pp,
                    w_bf["wo"][:, b, a * P : (a + 1) * P],
                    avT[:, b, :],
                    start=(b == 0),
                    stop=(b == ND - 1),
                )
            nc.vector.tensor_copy(poT_f[:, a, :], pp)

        # ---- gate gT = sigmoid(gw.T @ [qT; poT] + gb) ----
        gate_rhs = []
        for b in range(NG):
            src = qT_f[:, b, :] if b < ND else poT_f[:, b - ND, :]
            cb = pb_attn.tile([P, QS], BF16, name="gatecast", bufs=8)
            nc.vector.tensor_copy(cb, src)
            gate_rhs.append(cb)
        for a in range(ND):
            pp = pproj.tile([P, QS], F32, name="pproj_t")
            for b in range(NG):
                rhs = gate_rhs[b]
                nc.tensor.matmul(
                    pp,
                    w_bf["gw"][:, b, a * P : (a + 1) * P],
                    rhs,
                    start=(b == 0),
                    stop=(b == NG - 1),
                )
            nc.scalar.activation(
                out=gT[:, a, :], in_=pp, func=AF.Sigmoid, bias=gb[:, a : a + 1]
            )

        # ---- final combine: out = q + po + g*(q - po) ----
        for a in range(ND):
            s = pb_attn.tile([P, QS], F32, name="fin_t", bufs=6)
            nc.vector.tensor_tensor(
                out=s, in0=qT_f[:, a, :], in1=poT_f[:, a, :], op=OP.subtract
            )
            m = pb_attn.tile([P, QS], F32, name="fin_t", bufs=6)
            nc.vector.tensor_tensor(out=m, in0=gT[:, a, :], in1=s, op=OP.mult)
            r = pb_attn.tile([P, QS], F32, name="fin_t", bufs=6)
            nc.vector.tensor_tensor(
                out=r, in0=qT_f[:, a, :], in1=poT_f[:, a, :], op=OP.add
            )
            nc.vector.tensor_tensor(out=outT[:, a, :], in0=m, in1=r, op=OP.add)

        # ---- transpose back + one combined store ----
        out_nat = persist.tile([P, NI, D], F32, name="outn")
        for a in range(NI):
            pt = ptrans.tile([P, 4 * P], F32, name="ptrans_t")
            for b in range(ND):
                nc.tensor.transpose(
                    pt[:, b * P : (b + 1) * P],
                    outT[:, b, a * P : (a + 1) * P],
                    ident_f32,
                )
            nc.vector.tensor_copy(out_nat[:, a, :], pt)
        dst = out_d[:, :].rearrange("(a p) d -> p a d", p=P)
        nc.gpsimd.dma_start(out=dst, in_=out_nat)


_CACHE: dict = {}


def make_in_maps(inputs):
    """Shard full inputs into per-core input maps; returns (in_maps, flags)."""
    q = np.asarray(inputs["query"], np.float32)
    k = np.asarray(inputs["key"], np.float32)
    v = np.asarray(inputs["value"], np.float32)
    wq = np.asarray(inputs["weight_q"], np.float32)
    wk = np.asarray(inputs["weight_k"], np.float32)
    wv = np.asarray(inputs["weight_v"], np.float32)
    wo = np.asarray(inputs["weight_o"], np.float32)
    gw = np.asarray(inputs["g_w"], np.float32)
    gb = np.asarray(inputs["g_b"], np.float32)
    qmask = np.asarray(inputs["query_mask"])
    kmask = np.asarray(inputs["key_mask"])
    gams = [
        np.asarray(inputs[n], np.float32) for n in ("q_gamma", "k_gamma", "v_gamma")
    ]
    bets = [np.asarray(inputs[n], np.float32) for n in ("q_beta", "k_beta", "v_beta")]

    use_gamma = any(not np.allclose(g, 1.0) for g in gams)
    use_beta = any(np.any(bt != 0.0) for bt in bets)

    def colmajor(vec):  # [D] -> [128, ND] partition-major
        return np.ascontiguousarray(vec.reshape(-1, P).T)

    # padded K/V + per-key exp bias (0 = attend, -1e30 = masked)
    kpad = np.zeros((B, KP, D), np.float32)
    vpad = np.zeros((B, KP, D), np.float32)
    kpad[:, :KLEN] = k
    vpad[:, :KLEN] = v
    kmb = np.full((B, KP), NEGBIG, np.float32)
    kmb[:, :KLEN] = np.where(kmask == 0, NEGBIG, 0.0)
    kmb[:, KLEN] = 0.0  # zero-attn slot always attendable

    per_batch = NCORES // B
    in_maps = []
    for c in range(NCORES):
        b, r = c // per_batch, c % per_batch
        m = {
            "q": np.ascontiguousarray(q[b, r * QS : (r + 1) * QS]),
            "k": kpad[b],
            "v": vpad[b],
            "wq": wq,
            "wk": wk,
            "wv": wv,
            "wo": wo,
            "gw": gw,
            "gb": colmajor(gb),
            "kmb": np.ascontiguousarray(kmb[b].reshape(NJ, P).T),
            "qm": qmask[b, r * QS : (r + 1) * QS].astype(np.float32)[None, :],
        }
        if use_gamma:
            m["qg"], m["kg"], m["vg"] = (colmajor(g) for g in gams)
        if use_beta:
            m["qb"], m["kb"], m["vb"] = (colmajor(bt) for bt in bets)
        in_maps.append(m)
    return in_maps, (use_gamma, use_beta)


def kernel(_return_res=False, _run_kwargs=None, **inputs):
    run_kwargs = _run_kwargs or {}
    in_maps, key = make_in_maps(inputs)
    if key not in _CACHE:
        _CACHE[key] = _build(*key)
    nc = _CACHE[key]
    res = run_bass_kernel_spmd(nc, in_maps, list(range(NCORES)), **run_kwargs)
    out = np.empty((B, Q, D), np.float32)
    per_batch = NCORES // B
    for c in range(NCORES):
        b, r = c // per_batch, c % per_batch
        out[b, r * QS : (r + 1) * QS] = res.results[c]["out"]
    if _return_res:
        return out, res
    return out
